# revision 37
# baseline (speedup 1.0000x reference)
"""AllAtomFAPE loss on 8 TRN2 NeuronCores.

Strategy: dist^2[f,a] (+eps, masked) is a bilinear form over per-frame and
per-atom features:

    m_a^2 * (dist^2[f,a] + EPS) = sum_k A[a,k] * B[f,k],  K = 34

with A built from atom positions (pp/qq/pq outer products, p, q, 1) and B
from frame Gram matrices (Gp=RpRp^T, Gt=RtRt^T, M=RpRt^T, vectors, const).
The atom mask m^2 is folded into A (sqrt(m^2 x) = m sqrt(x)), EPS into B's
const row. Clamp folds through the sqrt: min(sqrt(y),10) = sqrt(min(y,100+EPS))
and max(y,0) guards rounding-induced negatives.

Frames (3072) are sharded across 8 cores (384 each); atoms replicated.
Per core: matmul (PE, bf16) -> clamp (DVE) -> sqrt + free-axis accumulate
(ACT accum_out, frame mask folded into the per-partition sqrt scale) ->
per-partition partial sums DMA'd out. Host sums the partials and applies
1/(atom_count*frame_count*Z). Masks are assumed 0/1 (AlphaFold semantics).
"""
import numpy as np
import ml_dtypes

import concourse.bass as bass
from concourse import bacc, tile, mybir
from concourse.bass_utils import run_bass_kernel_spmd

D_CLAMP = 10.0
EPS = 1e-4
Z = 10.0

B_, N_, F_, A_ = 1, 384, 8, 14
NF = N_ * F_            # 3072 frames total
NA = N_ * A_            # 5376 atoms
NCORES = 8
NF_LOC = NF // NCORES   # 384 frames per core
K = 34                  # bilinear contraction dim
FT = NF_LOC // 128      # 3 frame tiles per core
CHUNK = 512             # matmul free-dim cap

_cache = {}


# PSUM tiles: up to 2048 f32 = 4 banks (x2 bufs = all 8); matmul free-dim
# cap is 512. DVE clamps each PSUM tile into a big SBUF strip; ACT sqrt runs
# over big SBUF spans to amortize its ~352-cycle pipe + 187ns
# accumulator-read per-instruction overheads.
#  - ft0 leads with a small chunk so the DVE pipeline starts early
#  - the last ft uses per-chunk ACT so the final sqrt tail is short
DEFAULT_CFG = dict(   # TimelineSim-tuned chunk schedule
    ptiles_ft=[
        [(0, 640), (640, 1408), (2048, 2048), (4096, 1280)],
        [(0, 2048), (2048, 2048), (4096, 1280)],
        [(0, 2048), (2048, 1792), (3840, 1536)],
    ],
    act_ft=[
        [(0, 2048), (2048, 2048), (4096, 1280)],
        [(0, 2048), (2048, 2048), (4096, 1280)],
        [(0, 2048), (2048, 1792), (3840, 1536)],
    ],
    dma_stages=[(0, 640), (640, 1408), (2048, 2048), (4096, 1280)],
)


def _build_graph(cfg=None):
    cfg = cfg or DEFAULT_CFG
    nc = bacc.Bacc("TRN2", target_bir_lowering=False, debug=False)

    bf16 = mybir.dt.bfloat16
    f32 = mybir.dt.float32

    ptiles_ft = cfg["ptiles_ft"]
    act_ft = cfg["act_ft"]
    PW = max(cw for ft in ptiles_ft for (_, cw) in ft)
    assert PW <= 2048
    n_cols = sum(len(a) for a in act_ft)
    pcol = 0

    # pk packs bT (cols 0:NF_LOC) and A^T (cols NF_LOC:NF_LOC+NA) so the
    # first DMA delivers the stationary weights together with the first atom
    # chunk (one HWDGE transaction instead of two serialized ones).
    pk_d = nc.dram_tensor("pk", (K, NF_LOC + NA), bf16, kind="ExternalInput")
    fm_d = nc.dram_tensor("fm", (128, FT), f32, kind="ExternalInput")
    out_d = nc.dram_tensor("out", (128, n_cols), f32, kind="ExternalOutput")

    with tile.TileContext(nc) as tc:
        with (
            tc.tile_pool(name="const", bufs=1) as const,
            tc.tile_pool(name="big", bufs=2) as big,
            tc.tile_pool(name="psum", bufs=2, space="PSUM") as psum,
        ):
            pk = const.tile([K, NF_LOC + NA], bf16)
            fm = const.tile([128, FT], f32)
            partials = const.tile([128, n_cols], f32)

            # fm goes via the Pool engine's SWDGE path so it doesn't
            # serialize with pk on the single HWDGE queue
            nc.gpsimd.dma_start(out=fm[:], in_=fm_d[:])
            # stage the big pk load so the first matmuls start early; the
            # first slice carries bT plus the first atom columns
            for i, (a0, aw) in enumerate(cfg["dma_stages"]):
                c0 = 0 if i == 0 else NF_LOC + a0
                cw = NF_LOC + aw if i == 0 else aw
                nc.sync.dma_start(out=pk[:, c0:c0 + cw], in_=pk_d[:, c0:c0 + cw])

            for ft in range(FT):
                cl_big = big.tile([128, NA], f32, tag="cl")
                dummy = big.tile([128, 4096], bf16, tag="dummy")
                for (c0, cw) in ptiles_ft[ft]:
                    d2 = psum.tile([128, PW], f32)
                    for s0 in range(0, cw, CHUNK):
                        sw = min(CHUNK, cw - s0)
                        nc.tensor.matmul(
                            d2[:, s0:s0 + sw],
                            pk[:, ft * 128:(ft + 1) * 128],
                            pk[:, NF_LOC + c0 + s0:NF_LOC + c0 + s0 + sw],
                            start=True, stop=True,
                        )
                    nc.vector.tensor_scalar(
                        cl_big[:, c0:c0 + cw], d2[:, :cw],
                        0.0, 100.0 + EPS,
                        op0=mybir.AluOpType.max, op1=mybir.AluOpType.min,
                    )
                # sqrt(fm * y) = fm * sqrt(y) for 0/1 frame masks: the
                # per-partition scale folds the frame weighting into the
                # accumulation for free.
                for (a0, aw) in act_ft[ft]:
                    nc.scalar.activation(
                        dummy[:, :aw], cl_big[:, a0:a0 + aw],
                        mybir.ActivationFunctionType.Sqrt,
                        scale=fm[:, ft:ft + 1],
                        accum_out=partials[:, pcol:pcol + 1],
                    )
                    pcol += 1

            # final cross-partition/core reduction happens on the host:
            # just ship the (128, n) accumulator columns back
            nc.sync.dma_start(out=out_d[:, :pcol], in_=partials[:, :pcol])

    nc.compile()
    nc.finalize()
    return nc


def _features(predicted_frames_R, predicted_frames_t, predicted_atom_positions,
              atom_mask, true_frames_R, true_frames_t, true_atom_positions,
              seq_mask):
    """Host-side O(N+F) feature build. Returns A (NA,K), B (NF,K), fm, counts."""
    f32 = np.float32
    Rp = np.asarray(predicted_frames_R, f32).reshape(NF, 3, 3)
    tp = np.asarray(predicted_frames_t, f32).reshape(NF, 3)
    Rt = np.asarray(true_frames_R, f32).reshape(NF, 3, 3)
    tt = np.asarray(true_frames_t, f32).reshape(NF, 3)
    p = np.asarray(predicted_atom_positions, f32).reshape(NA, 3)
    q = np.asarray(true_atom_positions, f32).reshape(NA, 3)
    m = (np.asarray(atom_mask, f32) * np.asarray(seq_mask, f32)[:, :, None]).reshape(NA)
    fm = np.broadcast_to(
        np.asarray(seq_mask, f32)[:, :, None], (B_, N_, F_)).reshape(NF).copy()

    pp = np.einsum('aj,ak->ajk', p, p).reshape(NA, 9)
    qq = np.einsum('aj,ak->ajk', q, q).reshape(NA, 9)
    pq = np.einsum('aj,ak->ajk', p, q).reshape(NA, 9)
    Afeat = np.concatenate(
        [pp, qq, pq, p, q, np.ones((NA, 1), f32)], axis=1) * (m ** 2)[:, None]

    Gp = np.einsum('fij,fkj->fik', Rp, Rp)
    Gt = np.einsum('fij,fkj->fik', Rt, Rt)
    M = np.einsum('fij,fkj->fik', Rp, Rt)
    vec_p = -2 * np.einsum('fjk,fk->fj', Gp, tp) + 2 * np.einsum('fjk,fk->fj', M, tt)
    vec_q = -2 * np.einsum('fjk,fk->fj', Gt, tt) + 2 * np.einsum('fkj,fk->fj', M, tp)
    const = (np.einsum('fj,fjk,fk->f', tp, Gp, tp)
             + np.einsum('fj,fjk,fk->f', tt, Gt, tt)
             - 2 * np.einsum('fj,fjk,fk->f', tp, M, tt) + EPS)
    Bfeat = np.concatenate(
        [Gp.reshape(NF, 9), Gt.reshape(NF, 9), -2 * M.reshape(NF, 9),
         vec_p, vec_q, const[:, None]], axis=1)

    ac = max(float(m.sum()), 1.0)
    fc = max(float(fm.sum()), 1.0)
    return Afeat, Bfeat, fm, ac, fc


def make_in_maps(inputs):
    Afeat, Bfeat, fm, ac, fc = _features(**inputs)
    bf16 = ml_dtypes.bfloat16
    aT = Afeat.T.astype(bf16)                                  # (K, NA)
    in_maps = []
    for c in range(NCORES):
        Bc = Bfeat[c * NF_LOC:(c + 1) * NF_LOC]                # (NF_LOC, K)
        pk = np.concatenate([Bc.T.astype(bf16), aT], axis=1)   # (K, NF_LOC+NA)
        fmc = np.ascontiguousarray(
            fm[c * NF_LOC:(c + 1) * NF_LOC].reshape(FT, 128).T)  # (128, FT)
        in_maps.append({"pk": np.ascontiguousarray(pk), "fm": fmc})
    return in_maps, ac, fc


def kernel(**inputs) -> np.ndarray:
    in_maps, ac, fc = make_in_maps(inputs)

    if "nc" not in _cache:
        _cache["nc"] = _build_graph()
    nc = _cache["nc"]

    res = run_bass_kernel_spmd(nc, in_maps, core_ids=list(range(NCORES)))
    total = sum(float(r["out"].sum(dtype=np.float64)) for r in res.results)
    loss = total / (ac * fc * Z)
    return np.array([loss], np.float32)
